# revision 1
# baseline (speedup 1.0000x reference)
"""2-layer GAT (PyG GATConv x2 + linear + sigmoid) on 8 Trainium2 NeuronCores.

Self-contained: host-side graph preprocessing (degree-sorted node relabeling,
slot-bucketed dense edge layout, round-robin block sharding so one SPMD
program serves all 8 cores), a Bass/Tile kernel doing all FLOPs on device
(PE matmuls, indirect-DMA edge gathers, ACT/DVE softmax-aggregation,
AllReduce between layers), and a PJRT runner.

kernel(**inputs) takes the FULL unsharded inputs and returns the FULL
[50000, 2] float32 output.
"""

import sys

sys.path.insert(0, "/opt/trn_rl_repo")
import numpy as np
import concourse.bass as bass
from concourse import bacc
import concourse.tile as tile
from concourse import mybir
from concourse.bass import IndirectOffsetOnAxis
from concourse.masks import make_identity
from contextlib import ExitStack

F32 = mybir.dt.float32
I32 = mybir.dt.int32
AF = mybir.ActivationFunctionType
ALU = mybir.AluOpType

N = 50000
NC = 8
BLK = 128
NB = 392                 # 392 blocks of 128 = 50176 >= N
NBC = NB // NC           # 49 blocks per core
NPAD = NB * BLK          # 50176
SENT = NPAD              # sentinel row
V = NPAD + BLK           # table rows padded to 50304 (= 393*128) for flat zeroing
D1 = 136                 # h1(128) + al_s1(4) + al_d1(4)
D2 = 10                  # h2(8) + al_s2(1) + al_d2(1)
IN = 128
HEADS = 4
HID = 32
NEG = 0.2
EPS = 1e-16


def host_prep(edge_index):
    """Returns dict with permutation, slot tables per core, K list."""
    src = np.asarray(edge_index[0], dtype=np.int64)
    dst = np.asarray(edge_index[1], dtype=np.int64)
    deg = np.bincount(dst, minlength=NPAD).astype(np.int64)  # non-self degree (before loops)
    # order real+pad nodes by (self-inclusive) degree desc; pads (deg 0 w/o loop) last
    degloop = deg.copy()
    degloop[:N] += 1
    order = np.argsort(-degloop, kind="stable")       # newid -> origid
    rank = np.empty(NPAD, dtype=np.int64)
    rank[order] = np.arange(NPAD)                     # origid -> newid
    nsrc = rank[src]
    ndst = rank[dst]
    ndeg = np.bincount(ndst, minlength=NPAD).astype(np.int64)  # non-self degree by newid

    # per-position slot count: max non-self degree within stripe jj (blocks jj*8 .. jj*8+7)
    K_list = []
    for jj in range(NBC):
        lo, hi = (jj * NC) * BLK, (jj * NC + NC) * BLK
        K_list.append(max(1, int(ndeg[lo:hi].max())))
    K_arr = np.array(K_list)
    tot_slots = int(K_arr.sum())
    off = np.zeros(NBC, dtype=np.int64)
    off[1:] = np.cumsum(K_arr)[:-1]

    # build sidx[core][128, tot_slots] filled with SENT
    sidx = np.full((NC, BLK, tot_slots), SENT, dtype=np.int32)
    # per-edge slot position: k = rank of edge within its dst
    eo = np.argsort(ndst, kind="stable")
    sdst = ndst[eo]
    ssrc = nsrc[eo]
    starts = np.searchsorted(sdst, np.arange(NPAD))
    k_of = np.arange(len(sdst)) - starts[sdst]
    blk_of = sdst // BLK                 # global block id
    core_of = blk_of % NC
    jj_of = blk_of // NC
    row_of = sdst % BLK
    col_of = off[jj_of] + k_of
    # drop self-loop edges? self-loops were NOT in edge_index (we only count
    # the raw 800k edges here; self-loops handled separately on device).
    sidx[core_of, row_of, col_of] = ssrc.astype(np.int32)

    return dict(order=order, rank=rank, K_list=K_list, tot_slots=tot_slots,
                off=off, sidx=sidx, deg=ndeg)


def host_inputs(inp, prep):
    """Build per-core input maps from the raw problem inputs."""
    order = prep["order"]
    xT = np.zeros((IN, NPAD), dtype=np.float32)
    xv = np.asarray(inp["x"], dtype=np.float32)
    # col newid = x[order[newid]] for real nodes
    real = order < N
    xT[:, real] = xv[order[real]].T

    a_src1 = np.asarray(inp["a_src1"], np.float32)
    a_dst1 = np.asarray(inp["a_dst1"], np.float32)
    Ab1 = np.zeros((IN, 8), np.float32)
    for h in range(HEADS):
        Ab1[h*HID:(h+1)*HID, h] = a_src1[h]
        Ab1[h*HID:(h+1)*HID, 4+h] = a_dst1[h]
    Ab2 = np.zeros((8, 2), np.float32)
    Ab2[:, 0] = np.asarray(inp["a_src2"], np.float32)[0]
    Ab2[:, 1] = np.asarray(inp["a_dst2"], np.float32)[0]

    W1 = np.asarray(inp["W1"], np.float32)
    W2 = np.asarray(inp["W2"], np.float32)
    sent1 = np.zeros((1, D1), np.float32); sent1[0, 128:132] = -1e30
    sent2 = np.zeros((1, D2), np.float32); sent2[0, 8] = -1e30

    common = dict(
        xT=xT, W1=W1, W1T=np.ascontiguousarray(W1.T), Ab1=Ab1,
        W2=W2, W2T=np.ascontiguousarray(W2.T), Ab2=Ab2,
        b1=np.tile(np.asarray(inp["b1"], np.float32)[None, :], (BLK, 1)),
        b2=np.tile(np.asarray(inp["b2"], np.float32)[None, :], (BLK, 1)),
        Wl=np.asarray(inp["Wl"], np.float32),
        bl=np.tile(np.asarray(inp["bl"], np.float32)[None, :], (BLK, 1)),
        sent1=sent1, sent2=sent2,
    )
    maps = []
    for c in range(NC):
        m = dict(common)
        m["sidx"] = prep["sidx"][c]
        maps.append(m)
    return maps


def bcast_free(ap_obj, n):
    """Append a step-0 free dim of size n to an AP."""
    return bass.AP(ap_obj.tensor, ap_obj.offset, list(ap_obj.ap) + [[0, n]])


def build(K_list, tot_slots, phase=3, reps=1, dbg_block=0):
    nc = bacc.Bacc("TRN2", target_bir_lowering=False, debug=False,
                   enable_asserts=True, num_devices=NC)
    off = np.zeros(NBC, dtype=np.int64)
    off[1:] = np.cumsum(np.array(K_list))[:-1]

    xT = nc.dram_tensor("xT", [IN, NPAD], F32, kind="ExternalInput").ap()
    W1 = nc.dram_tensor("W1", [IN, IN], F32, kind="ExternalInput").ap()
    W1T = nc.dram_tensor("W1T", [IN, IN], F32, kind="ExternalInput").ap()
    Ab1 = nc.dram_tensor("Ab1", [IN, 8], F32, kind="ExternalInput").ap()
    W2 = nc.dram_tensor("W2", [IN, 8], F32, kind="ExternalInput").ap()
    W2T = nc.dram_tensor("W2T", [8, IN], F32, kind="ExternalInput").ap()
    Ab2 = nc.dram_tensor("Ab2", [8, 2], F32, kind="ExternalInput").ap()
    b1 = nc.dram_tensor("b1", [BLK, IN], F32, kind="ExternalInput").ap()
    b2 = nc.dram_tensor("b2", [BLK, 8], F32, kind="ExternalInput").ap()
    Wl = nc.dram_tensor("Wl", [8, 2], F32, kind="ExternalInput").ap()
    bl = nc.dram_tensor("bl", [BLK, 2], F32, kind="ExternalInput").ap()
    sent1 = nc.dram_tensor("sent1", [1, D1], F32, kind="ExternalInput").ap()
    sent2 = nc.dram_tensor("sent2", [1, D2], F32, kind="ExternalInput").ap()
    sidx = nc.dram_tensor("sidx", [BLK, tot_slots], I32, kind="ExternalInput").ap()
    cid = nc.dram_tensor("cid", [1, 1], I32, kind="ExternalInput").ap()  # core id 0..7

    hext1 = nc.dram_tensor("hext1", [V, D1], F32).ap()
    h2part = nc.dram_tensor("h2part", [V, D2], F32).ap()
    hext2 = nc.dram_tensor("hext2", [V, D2], F32, addr_space="Shared").ap()

    outp = nc.dram_tensor("outp", [NBC * BLK, 2], F32, kind="ExternalOutput").ap()
    dbg = nc.dram_tensor("dbg", [BLK, D1 + 16], F32, kind="ExternalOutput").ap()

    CHUNK = 512
    NCHUNK = NPAD // CHUNK

    with tile.TileContext(nc, trace_sim=False) as tc, ExitStack() as ctx:
        const = ctx.enter_context(tc.tile_pool(name="const", bufs=1))
        ps1 = ctx.enter_context(tc.tile_pool(name="ps1", bufs=1, space="PSUM"))
        psh = ctx.enter_context(tc.tile_pool(name="psh", bufs=2, space="PSUM"))
        sb = ctx.enter_context(tc.tile_pool(name="sb", bufs=4))
        gpool = ctx.enter_context(tc.tile_pool(name="gp", bufs=2))
        mpool = ctx.enter_context(tc.tile_pool(name="mp", bufs=2))

        def LD(name, apx, shp, dt=F32):
            t = const.tile(shp, dt, tag=name)
            nc.sync.dma_start(out=t[:], in_=apx)
            return t

        W1_t = LD("W1", W1, [IN, IN]); W1T_t = LD("W1T", W1T, [IN, IN])
        Ab1_t = LD("Ab1", Ab1, [IN, 8]); W2_t = LD("W2", W2, [IN, 8])
        W2T_t = LD("W2T", W2T, [8, IN]); Ab2_t = LD("Ab2", Ab2, [8, 2])
        b1_t = LD("b1", b1, [BLK, IN]); b2_t = LD("b2", b2, [BLK, 8])
        Wl_t = LD("Wl", Wl, [8, 2]); bl_t = LD("bl", bl, [BLK, 2])
        s1_t = LD("s1", sent1, [1, D1]); s2_t = LD("s2", sent2, [1, D2])
        sidx_t = LD("sidx", sidx, [BLK, tot_slots], I32)
        cid_t = LD("cid", cid, [1, 1], I32)
        ident = const.tile([128, 128], F32, tag="ident")
        make_identity(nc, ident[:])

        # core id into a register (for per-core block base addressing)
        creg = nc.sync.value_load(cid_t[0:1, 0:1])

        for rep in range(reps):
            # ---------- Phase A: hext1 ----------
            w1a_ps = ps1.tile([IN, 8], F32, tag="wa")
            nc.tensor.matmul(w1a_ps[:], lhsT=W1T_t[:], rhs=Ab1_t[:], start=True, stop=True)
            W1ext_t = const.tile([IN, D1], F32, tag=f"w1ext{rep}")
            nc.scalar.copy(W1ext_t[:, 0:IN], W1_t[:])
            nc.scalar.copy(W1ext_t[:, IN:D1], w1a_ps[:])

            w2a_ps = ps1.tile([IN, 2], F32, tag="wa")
            nc.tensor.matmul(w2a_ps[:], lhsT=W2T_t[:], rhs=Ab2_t[:], start=True, stop=True)
            W2ext_t = const.tile([IN, D2], F32, tag=f"w2ext{rep}")
            nc.scalar.copy(W2ext_t[:, 0:8], W2_t[:])
            nc.scalar.copy(W2ext_t[:, 8:10], w2a_ps[:])

            for ci in range(NCHUNK):
                xt = sb.tile([IN, CHUNK], F32, tag="xt")
                nc.sync.dma_start(out=xt[:], in_=xT[:, ci*CHUNK:(ci+1)*CHUNK])
                for sub in range(CHUNK // 128):
                    hp = psh.tile([128, D1], F32, tag="hp")
                    nc.tensor.matmul(hp[:], lhsT=xt[:, sub*128:(sub+1)*128],
                                     rhs=W1ext_t[:], start=True, stop=True)
                    hs = sb.tile([128, D1], F32, tag="hs")
                    if sub % 2 == 0:
                        nc.scalar.copy(hs[:], hp[:])
                    else:
                        nc.vector.tensor_copy(hs[:], hp[:])
                    r0 = ci * CHUNK + sub * 128
                    nc.sync.dma_start(out=hext1[r0:r0+128, :], in_=hs[:])
            nc.sync.dma_start(out=hext1[SENT:SENT+1, :], in_=s1_t[:])
            # zero pad rows SENT+1 .. V
            zpad = sb.tile([BLK - 1, D1], F32, tag="zpad")
            nc.vector.memset(zpad[:], 0.0)
            nc.sync.dma_start(out=hext1[SENT+1:V, :], in_=zpad[:])

            if phase < 2:
                continue
            tc.strict_bb_all_engine_barrier()

            # zero h2part (flat [128, V*D2/128])
            zw = V * D2 // 128
            zt = sb.tile([128, zw], F32, tag="zt")
            nc.vector.memset(zt[:], 0.0)
            nc.sync.dma_start(
                out=h2part.rearrange("(a b) d -> a (b d)", a=128), in_=zt[:])
            tc.strict_bb_all_engine_barrier()
            nc.sync.dma_start(out=h2part[SENT:SENT+1, :], in_=s2_t[:])

            # ---------- Phase B: layer-1 blocks ----------
            for jj in range(NBC):
                K = K_list[jj]
                # block base row = ((jj*8 + core)*128) -> dynamic via register
                base = (creg + jj * NC) * BLK

                Gd = gpool.tile([BLK, D1], F32, tag="Gd")
                nc.sync.dma_start(out=Gd[:], in_=hext1[bass.ds(base, BLK), :])

                G = gpool.tile([BLK, K, D1], F32, tag="G")
                for k in range(K):
                    s = int(off[jj]) + k
                    nc.gpsimd.indirect_dma_start(
                        out=G[:, k, :], out_offset=None, in_=hext1,
                        in_offset=IndirectOffsetOnAxis(ap=sidx_t[:, s:s+1], axis=0))

                P = mpool.tile([BLK, K, HEADS], F32, tag="P")
                Pself = mpool.tile([BLK, HEADS], F32, tag="Pself")
                for h in range(HEADS):
                    ald_h = Gd[:, 132+h:133+h]
                    nc.scalar.activation(P[:, :, h], G[:, :, 128+h], AF.Identity,
                                         bias=ald_h, scale=1.0)
                    nc.scalar.activation(Pself[:, h:h+1], Gd[:, 128+h:129+h],
                                         AF.Identity, bias=ald_h, scale=1.0)
                nc.vector.scalar_tensor_tensor(out=P[:], in0=P[:], scalar=NEG,
                                               in1=P[:], op0=ALU.mult, op1=ALU.max)
                nc.vector.scalar_tensor_tensor(out=Pself[:], in0=Pself[:], scalar=NEG,
                                               in1=Pself[:], op0=ALU.mult, op1=ALU.max)
                nc.scalar.activation(P[:], P[:], AF.Exp)
                nc.scalar.activation(Pself[:], Pself[:], AF.Exp)

                den = mpool.tile([BLK, HEADS], F32, tag="den")
                nc.vector.tensor_reduce(den[:], P[:].rearrange("p k h -> p h k"),
                                        axis=mybir.AxisListType.X, op=ALU.add)
                nc.vector.tensor_tensor(den[:], den[:], Pself[:], op=ALU.add)
                r = mpool.tile([BLK, HEADS], F32, tag="r")
                nc.vector.tensor_scalar_add(r[:], den[:], EPS)
                nc.vector.reciprocal(r[:], r[:])

                M = mpool.tile([BLK, K, IN], F32, tag="M")
                nc.vector.tensor_tensor(
                    M[:].rearrange("p k (h j) -> p k h j", h=HEADS),
                    G[:, :, 0:IN].rearrange("p k (h j) -> p k h j", h=HEADS),
                    bcast_free(P[:], HID), op=ALU.mult)
                agg = mpool.tile([BLK, IN], F32, tag="agg")
                nc.vector.tensor_reduce(agg[:], M[:].rearrange("p k f -> p f k"),
                                        axis=mybir.AxisListType.X, op=ALU.add)
                selfm = mpool.tile([BLK, IN], F32, tag="selfm")
                nc.vector.tensor_tensor(
                    selfm[:].rearrange("p (h j) -> p h j", h=HEADS),
                    Gd[:, 0:IN].rearrange("p (h j) -> p h j", h=HEADS),
                    bcast_free(Pself[:], HID), op=ALU.mult)
                nc.vector.tensor_tensor(agg[:], agg[:], selfm[:], op=ALU.add)
                # normalize + bias + ELU
                outb = mpool.tile([BLK, IN], F32, tag="outb")
                nc.vector.tensor_tensor(
                    outb[:].rearrange("p (h j) -> p h j", h=HEADS),
                    agg[:].rearrange("p (h j) -> p h j", h=HEADS),
                    bcast_free(r[:], HID), op=ALU.mult)
                nc.vector.tensor_tensor(outb[:], outb[:], b1_t[:], op=ALU.add)
                neg = mpool.tile([BLK, IN], F32, tag="neg")
                nc.vector.tensor_scalar_min(neg[:], outb[:], 0.0)
                nc.scalar.activation(neg[:], neg[:], AF.Exp)
                elu = mpool.tile([BLK, IN], F32, tag="elu")
                nc.vector.scalar_tensor_tensor(
                    out=elu[:], in0=outb[:], scalar=0.0, in1=neg[:],
                    op0=ALU.max, op1=ALU.add)
                nc.vector.tensor_scalar_add(elu[:], elu[:], -1.0)
                # h2|al2 = elu @ W2ext  (via PE transpose)
                eT_ps = ps1.tile([128, 128], F32, tag="tr")
                nc.tensor.transpose(eT_ps[:], elu[:], ident[:])
                eT = mpool.tile([128, 128], F32, tag="eTs")
                nc.vector.tensor_copy(eT[:], eT_ps[:])
                o2_ps = ps1.tile([BLK, D2], F32, tag="o2")
                nc.tensor.matmul(o2_ps[:], lhsT=eT[:], rhs=W2ext_t[:], start=True, stop=True)
                o2 = mpool.tile([BLK, D2], F32, tag="o2s")
                nc.scalar.copy(o2[:], o2_ps[:])
                nc.sync.dma_start(out=h2part[bass.ds(base, BLK), :], in_=o2[:])

                if phase < 3 and jj == dbg_block:
                    dbt = sb.tile([BLK, D1 + 16], F32, tag="dbt")
                    nc.vector.memset(dbt[:], 0.0)
                    nc.vector.tensor_copy(dbt[:, 0:D2], o2[:])
                    nc.vector.tensor_copy(dbt[:, 16:16+HEADS], den[:])
                    nc.vector.tensor_copy(dbt[:, 24:24+IN], outb[:])
                    nc.sync.dma_start(out=dbg, in_=dbt[:])

            if phase < 3:
                continue

            tc.strict_bb_all_engine_barrier()
            # ---------- AllReduce hext2 ----------
            nc.gpsimd.collective_compute(
                "AllReduce", ALU.add, replica_groups=[list(range(NC))],
                ins=[h2part.opt()], outs=[hext2.opt()])

            tc.strict_bb_all_engine_barrier()
            # ---------- Phase C: layer-2 blocks + final ----------
            for jj in range(NBC):
                K = K_list[jj]
                base = (creg + jj * NC) * BLK
                Gd2 = gpool.tile([BLK, D2], F32, tag="Gd2")
                nc.sync.dma_start(out=Gd2[:], in_=hext2[bass.ds(base, BLK), :])
                G2 = gpool.tile([BLK, K, D2], F32, tag="G2")
                for k in range(K):
                    s = int(off[jj]) + k
                    nc.gpsimd.indirect_dma_start(
                        out=G2[:, k, :], out_offset=None, in_=hext2,
                        in_offset=IndirectOffsetOnAxis(ap=sidx_t[:, s:s+1], axis=0))

                P2 = mpool.tile([BLK, K], F32, tag="P2")
                P2s = mpool.tile([BLK, 1], F32, tag="P2s")
                ald2 = Gd2[:, 9:10]
                nc.scalar.activation(P2[:], G2[:, :, 8], AF.Identity,
                                     bias=ald2, scale=1.0)
                nc.scalar.activation(P2s[:], Gd2[:, 8:9], AF.Identity,
                                     bias=ald2, scale=1.0)
                nc.vector.scalar_tensor_tensor(out=P2[:], in0=P2[:], scalar=NEG,
                                               in1=P2[:], op0=ALU.mult, op1=ALU.max)
                nc.vector.scalar_tensor_tensor(out=P2s[:], in0=P2s[:], scalar=NEG,
                                               in1=P2s[:], op0=ALU.mult, op1=ALU.max)
                nc.scalar.activation(P2[:], P2[:], AF.Exp)
                nc.scalar.activation(P2s[:], P2s[:], AF.Exp)
                den2 = mpool.tile([BLK, 1], F32, tag="den2")
                nc.vector.tensor_reduce(den2[:], P2[:], axis=mybir.AxisListType.X, op=ALU.add)
                nc.vector.tensor_tensor(den2[:], den2[:], P2s[:], op=ALU.add)
                r2 = mpool.tile([BLK, 1], F32, tag="r2")
                nc.vector.tensor_scalar_add(r2[:], den2[:], EPS)
                nc.vector.reciprocal(r2[:], r2[:])

                M2 = mpool.tile([BLK, K, 8], F32, tag="M2")
                nc.vector.tensor_tensor(
                    M2[:], G2[:, :, 0:8],
                    bcast_free(P2[:], 8),
                    op=ALU.mult)
                agg2 = mpool.tile([BLK, 8], F32, tag="agg2")
                nc.vector.tensor_reduce(agg2[:], M2[:].rearrange("p k f -> p f k"),
                                        axis=mybir.AxisListType.X, op=ALU.add)
                self2 = mpool.tile([BLK, 8], F32, tag="self2")
                nc.vector.tensor_tensor(self2[:], Gd2[:, 0:8],
                                        bcast_free(P2s[:].squeeze(1), 8),
                                        op=ALU.mult)
                nc.vector.tensor_tensor(agg2[:], agg2[:], self2[:], op=ALU.add)
                outb2 = mpool.tile([BLK, 8], F32, tag="outb2")
                nc.vector.tensor_tensor(outb2[:], agg2[:],
                                        bcast_free(r2[:].squeeze(1), 8),
                                        op=ALU.mult)
                nc.vector.tensor_tensor(outb2[:], outb2[:], b2_t[:], op=ALU.add)
                neg2 = mpool.tile([BLK, 8], F32, tag="neg2")
                nc.vector.tensor_scalar_min(neg2[:], outb2[:], 0.0)
                nc.scalar.activation(neg2[:], neg2[:], AF.Exp)
                elu2 = mpool.tile([BLK, 8], F32, tag="elu2")
                nc.vector.scalar_tensor_tensor(
                    out=elu2[:], in0=outb2[:], scalar=0.0, in1=neg2[:],
                    op0=ALU.max, op1=ALU.add)
                nc.vector.tensor_scalar_add(elu2[:], elu2[:], -1.0)
                # final: sigmoid(elu2 @ Wl + bl)
                e2T_ps = ps1.tile([8, 128], F32, tag="tr")
                nc.tensor.transpose(e2T_ps[:], elu2[:], ident[:])
                e2T = mpool.tile([8, 128], F32, tag="e2Ts")
                nc.vector.tensor_copy(e2T[:], e2T_ps[:])
                fin_ps = ps1.tile([BLK, 2], F32, tag="fin")
                nc.tensor.matmul(fin_ps[:], lhsT=e2T[:], rhs=Wl_t[:], start=True, stop=True)
                fin = mpool.tile([BLK, 2], F32, tag="fins")
                nc.vector.tensor_tensor(fin[:], fin_ps[:], bl_t[:], op=ALU.add)
                nc.scalar.activation(fin[:], fin[:], AF.Sigmoid)
                nc.sync.dma_start(out=outp[jj*BLK:(jj+1)*BLK, :], in_=fin[:])

        if phase < 2:
            dbt = sb.tile([BLK, D1 + 16], F32, tag="dbt")
            nc.vector.memset(dbt[:], 0.0)
            nc.sync.dma_start(out=dbt[:, 0:D1], in_=hext1[dbg_block*128:(dbg_block+1)*128, :])
            nc.sync.dma_start(out=dbg, in_=dbt[:])
        if phase < 3:
            zo = sb.tile([128, NBC * 2], F32, tag="zo")
            nc.vector.memset(zo[:], 0.0)
            nc.sync.dma_start(out=outp.rearrange("(a b) d -> a (b d)", a=128), in_=zo[:])
        else:
            dbt = sb.tile([BLK, D1 + 16], F32, tag="dbt")
            nc.vector.memset(dbt[:], 0.0)
            nc.sync.dma_start(out=dbg, in_=dbt[:])
    nc.compile()
    return nc


# ----------------------------------------------------------------------------
# PJRT runner (upload once, execute once)
# ----------------------------------------------------------------------------
import jax
from jax.sharding import Mesh, PartitionSpec, NamedSharding
from jax.experimental.shard_map import shard_map
from concourse import bass2jax
from concourse.bass2jax import _bass_exec_p, partition_id_tensor, install_neuronx_cc_hook
from concourse.bass_interp import get_hw_module


def make_runner(nc, in_maps, n_cores=8, donate=False):

    install_neuronx_cc_hook()
    hw_m = get_hw_module(nc.m)
    old_m = nc.m
    nc.m = hw_m

    partition_name = nc.partition_id_tensor.name if nc.partition_id_tensor else None
    in_names, out_names, out_avals, zero_outs = [], [], [], []
    for alloc in nc.m.functions[0].allocations:
        if not isinstance(alloc, mybir.MemoryLocationSet):
            continue
        name = alloc.memorylocations[0].name
        if alloc.kind == "ExternalInput":
            if name != partition_name:
                in_names.append(name)
        elif alloc.kind == "ExternalOutput":
            out_names.append(name)
            shape = tuple(alloc.tensor_shape)
            dtype = mybir.dt.np(alloc.dtype)
            out_avals.append(jax.core.ShapedArray(shape, dtype))
            zero_outs.append(np.zeros(shape, dtype))
    n_params = len(in_names)
    n_outs = len(out_avals)
    all_in_names = list(in_names) + list(out_names)
    if partition_name is not None:
        all_in_names_full = all_in_names + [partition_name]
    else:
        all_in_names_full = all_in_names

    def _body(*args):
        operands = list(args)
        if partition_name is not None:
            operands.append(partition_id_tensor())
        outs = _bass_exec_p.bind(
            *operands,
            out_avals=tuple(out_avals),
            in_names=tuple(all_in_names_full),
            out_names=tuple(out_names),
            lowering_input_output_aliases=(),
            sim_require_finite=True,
            sim_require_nnan=True,
            nc=nc,
        )
        return tuple(outs)

    devices = jax.devices()[:n_cores]
    mesh = Mesh(np.asarray(devices), ("core",))
    in_specs = (PartitionSpec("core"),) * (n_params + n_outs)
    out_specs = (PartitionSpec("core"),) * n_outs
    jit_kwargs = dict(keep_unused=True)
    if donate:
        jit_kwargs["donate_argnums"] = tuple(range(n_params, n_params + n_outs))
    sharded = jax.jit(
        shard_map(_body, mesh=mesh, in_specs=in_specs, out_specs=out_specs, check_rep=False),
        **jit_kwargs,
    )
    per_core = [[np.asarray(m[name]) for name in in_names] for m in in_maps]
    concat_in = [
        np.concatenate([per_core[c][i] for c in range(n_cores)], axis=0)
        for i in range(n_params)
    ]
    concat_zeros = [np.zeros((n_cores * z.shape[0], *z.shape[1:]), z.dtype) for z in zero_outs]
    sharding = NamedSharding(mesh, PartitionSpec("core"))
    dev_in = [jax.device_put(a, sharding) for a in concat_in]
    dev_zeros = [jax.device_put(a, sharding) for a in concat_zeros]
    jax.block_until_ready(dev_in)

    state = {"nc_m_restored": False}

    def run():
        outs = sharded(*dev_in, *dev_zeros)
        jax.block_until_ready(outs)
        return outs

    def results_of(outs):
        return [
            {name: np.asarray(outs[i]).reshape(n_cores, *out_avals[i].shape)[c]
             for i, name in enumerate(out_names)}
            for c in range(n_cores)
        ]

    nc.m = old_m
    return run, results_of


_CACHE = {}


def kernel(**inputs):
    import numpy as np
    edge_index = np.asarray(inputs["edge_index"])
    prep = host_prep(edge_index)
    maps = host_inputs(inputs, prep)
    for c in range(NC):
        maps[c]["cid"] = np.array([[c]], dtype=np.int32)
    key = tuple(prep["K_list"])
    if key not in _CACHE:
        nc = build(prep["K_list"], prep["tot_slots"], phase=3)
        run, results_of = make_runner(nc, maps, n_cores=NC, donate=False)
        _CACHE[key] = (run, results_of)
    run, results_of = _CACHE[key]
    res = results_of(run())
    order = prep["order"]
    full = np.zeros((NPAD, 2), np.float32)
    for c in range(NC):
        o = res[c]["outp"]
        for jj in range(NBC):
            full[(jj * NC + c) * BLK:(jj * NC + c) * BLK + BLK] = o[jj*BLK:(jj+1)*BLK]
    real = order < N
    result = np.zeros((N, 2), np.float32)
    result[order[real]] = full[np.arange(NPAD)[real]]
    return result

